# revision 76
# baseline (speedup 1.0000x reference)
"""AttentionBlock (GroupNorm + single-head self-attention + proj + residual)
on 8 TRN2 NeuronCores. Data-parallel over batch: core i handles sample i.

Reference computation per sample (C=256, H=W=64, N=H*W=4096, G=32 groups):
  h    = groupnorm(x) * gamma + beta
  qkv  = w_qkv @ h + b_qkv              (1x1 conv == channel matmul)
  attn = softmax(q^T k / sqrt(C))       (N x N, never materialized in HBM)
  out  = x + w_proj @ (v @ attn^T) + b_proj

v4 structure (~209us, vs 226us for v3 and 257us for the first version):
  - w_proj folded into the v projection on the host: vp = (w_proj@w_v) h,
    so attn@v directly produces projected channels. out = x+(E vp)/den+bp.
  - GroupNorm folded into the qkv weights on device (w' = w * sc per input
    channel), so x casts to fp8 on arrival and no h tensor exists. The
    fold is staggered (k cols on DVE first, vp next, q on ACT) so the
    first projections start as soon as their columns are folded.
  - bias algebra: k needs no bias (cancels in softmax); vp needs none (a
    constant rides into bp_eff); q's [P,1] bias comes from 8 tiny fp8
    matmuls against wq8 with rhs 16*sh/sc (wqf fp32 LDWEIGHTS cost 333ns
    each on the startup critical path).
  - exps split WITHIN each pair: half0 on ACT (exact Exp), half1 on the
    custom DVE op ((x*c0+1)^2*c1+c2)^8 ~= e^(x/16) (1.2% rel err). One
    686ns exp per engine per 1075ns gp -- runs of 2 pairs per engine are
    structurally 400-700ns late, single-pair runs leave no queue slack.
    Pairs p4/p8 go fully to ACT so the divide's DVE work (in half-width
    chunks at p4/5/8/9) lands in DVE exp-free slots.
  - x loads as 2048-col 8KB-descriptor chunks (~143GB/s/ring vs 79 at
    4KB); the slow scalar ring carries the middle columns before wqf.
    bn_stats/casts chase chunk arrival; the whole stats->fold->bias chain
    is merged into one [P,2]-column-pair pass.
  - kv blocks 1-7 stream through block 0 as 2-matmul pieces (one psum->
    fp8 cast per engine per iteration, emitted AFTER the gp's exps);
    pre-attention only k(0)+bias+q(0) sit ahead of the first score.
  - q projections for blocks 2..7 are deferred into attention blocks 1..6.
  - wq8 padded +16 cols: 768-stride fp8 LDWEIGHTS took 370ns vs 134 at
    vt's 272 stride.
  - the last block's divide/store runs in half-width chunks fanned across
    ACT/GPS/DVE and 3 DMA queues; its den matmul is emitted first.
  - x stays resident in SBUF for the residual (no 4MB re-read).
"""

import sys

for _p in ("/opt/trn_rl_repo", "/opt/pypackages"):
    if _p not in sys.path:
        sys.path.append(_p)

from contextlib import ExitStack

import numpy as np

import concourse.bass as bass
import concourse.tile as tile
from concourse import bacc, mybir
from concourse._compat import with_exitstack

B, C, H, W = 8, 256, 64, 64
N = H * W          # 4096
G = 32             # groups
GS = C // G        # 8 channels per group
EPS = 1e-5
P = 128
NCT = C // P       # 2 channel tiles
NBLK = 512         # attention n-block width
NB = N // NBLK     # 8
NM = N // P        # 32 m-tiles
SCALE = 1.0 / np.sqrt(np.float32(C))  # 1/16
WARMUP_MM = 52      # fp32 gmat matmuls to keep PE's HAM clock-gate warm

F32 = mybir.dt.float32
BF16 = mybir.dt.bfloat16
FP8 = mybir.dt.float8e4
DR = mybir.MatmulPerfMode.DoubleRow
AF = mybir.ActivationFunctionType
ALU = mybir.AluOpType

# ---- custom DVE op: out = ((x*c0 + 1)^2 * c1 + c2)^8 ~= exp(x * 8*c0) ----
# Exactly fills the v3 pipeline's 8 ALU stages; constants minimax-fitted so
# that with c0 = a*SCALE the op approximates exp(x*SCALE) to 1.2% rel err
# for |x*SCALE| <= 2.6 (scaled scores are ~N(0, 0.4^2); per-sample max |s|
# ~2.3). The fp8 output rounding (~3%) dominates this.
_EXP_A = 0.12251085
_EXP_C1 = 0.51681271
_EXP_C2 = 0.4835532
EXP_C0 = float(_EXP_A * SCALE)


def _exp8_ref(in0, in1, s0, s1, imm2):
    u = in0.astype(np.float32) * s0 + 1.0
    w = u * u * s1 + imm2
    t = w * w
    t = t * t
    return t * t


def _register_exp8():
    import concourse.dve_ops as dve_ops
    from concourse.dve_ops import DveOp
    from concourse.dve_spec import C0, C1, C2, One, Spec, Src0
    from concourse.dve_spec import lower as dve_lower
    from concourse.dve_uop import DveOpSpec

    if any(op.name == "EXP8_ANT" for op in dve_ops.OPS):
        return next(op for op in dve_ops.OPS if op.name == "EXP8_ANT")
    body = Src0 * C0 + One
    body = body * body
    body = body * C1 + C2
    body = body * body
    body = body * body
    body = body * body
    spec = Spec(body=body, reference=_exp8_ref)
    row = max(dve_ops._SUB_OPCODE_FOR_NAME.values()) + 1
    assert row < 0x20
    sha = {
        ver: DveOpSpec(
            name="EXP8_ANT", opcode=row, uops=dve_lower(spec, ver=ver),
            rd1_en=False,
        ).sha(ver)
        for ver in ("v3",)
    }
    op = DveOp("EXP8_ANT", spec, subdim=False, uops_sha=sha)
    dve_ops.OPS.append(op)
    dve_ops.CUSTOM_DVE_SPECS[op.name] = spec
    dve_ops._SUB_OPCODE_FOR_NAME[op.name] = row
    return op


EXP8 = _register_exp8()


def _group_mat() -> np.ndarray:
    """A[c, c'] = 1/GS if c and c' are in the same group (within a 128-chan
    tile); A^T @ t group-averages per-channel stats in one PE matmul."""
    a = np.zeros((P, P), np.float32)
    for g in range(P // GS):
        a[g * GS:(g + 1) * GS, g * GS:(g + 1) * GS] = 1.0 / GS
    return a


@with_exitstack
def emit_kernel(ctx: ExitStack, tc: tile.TileContext, out_d, x_d, wqkvT_d,
                consts_d, gmat_d):
    nc = tc.nc

    big = ctx.enter_context(tc.tile_pool(name="big", bufs=1))
    small = ctx.enter_context(tc.tile_pool(name="small", bufs=1))
    work = ctx.enter_context(tc.tile_pool(name="work", bufs=3))
    work2 = ctx.enter_context(tc.tile_pool(name="work2", bufs=3))
    tdiv = ctx.enter_context(tc.tile_pool(name="tdiv", bufs=4))
    stage = ctx.enter_context(tc.tile_pool(name="stage", bufs=4))
    ps_s = ctx.enter_context(tc.tile_pool(name="ps_s", bufs=3, space="PSUM"))
    ps_av0 = ctx.enter_context(tc.tile_pool(name="ps_av0", bufs=2, space="PSUM"))
    ps_av1 = ctx.enter_context(tc.tile_pool(name="ps_av1", bufs=2, space="PSUM"))
    ps_sum = ctx.enter_context(tc.tile_pool(name="ps_sum", bufs=1, space="PSUM"))

    # ---- scalar queue first: gmat (64KB, feeds PE warmups at ~9us), the
    # packed consts [128, 8] (gamma|beta|bp|bq -- eight separate [128,1]
    # column DMAs cost ~1.1us of issue each), then wqf. All issued before
    # the sqrt preload so its ACT table load can't delay them. ----
    gmat_f = small.tile([P, P], F32, tag="gmatf")
    nc.scalar.dma_start(gmat_f, gmat_d[:, :])
    consts_t = small.tile([P, 8], F32, tag="consts")
    nc.scalar.dma_start(consts_t, consts_d[:, :])
    # consts cols: gamma0 gamma1 beta0 beta1 bq0 bq1 bp0 bp1
    # (wqf is DMA'd from the load section below, AFTER the scalar ring's
    # x middle-columns: it is only needed at fold time)

    gmat_sb = small.tile([P, P], F32, tag="gmat")
    nc.vector.tensor_copy(gmat_sb, gmat_f)

    eps_t = small.tile([P, 1], F32, tag="eps")
    nc.vector.memset(eps_t, float(EPS))
    # preload the Sqrt act table while the engines boot (Sqrt and Exp live
    # in different table sets; each implicit load costs 1.28us on ACT)
    sqrt_dummy = small.tile([P, 1], F32, tag="sqrt_dummy")
    nc.scalar.activation(sqrt_dummy, eps_t, AF.Sqrt, bias=eps_t)

    # ---- load x balanced over all 3 DMA rings (measured: ~125-150GB/s
    # per ring with 8KB per-partition descriptors, ~79 with 4KB). Each
    # ring carries ~1.4-1.5MB: one big 2048-col chunk first, then a
    # 1024-col chunk of the other channel-tile; the scalar ring follows
    # its gmat/consts/wqf with the 3072..4095 columns. bn_stats (DVE) +
    # fp8 cast (ACT) are emitted in expected ARRIVAL order (j 0-3, 6-7,
    # 4-5) so the tail stats don't queue behind not-yet-arrived data. ----
    x_sb = big.tile([P, 2, N], F32, tag="x", name="x")
    stats_t = []
    for ct in range(NCT):
        stats_t.append(small.tile([P, NB, 6], F32, tag=f"bnst{ct}",
                                  name=f"bnst{ct}"))
    x8 = big.tile([P, 2, N], FP8, tag="x8")
    # Rings sustain ~140-150GB/s regardless of descriptor size >= 4KB (a
    # single full-width 16KB-desc DMA was no faster and starved the stats
    # pipeline of early data). Schedule so every chunk lands BEFORE the
    # serial 10.9us DVE stats pipeline reaches it: big leading chunks on
    # sync/gpsimd, the middle columns early on the scalar ring (ahead of
    # wqf, which is only needed at fold time).
    def ld(eng, ct, c0, c1):
        eng.dma_start(x_sb[:, ct, c0:c1], x_d[ct * P:ct * P + P, c0:c1])

    # ~1.6MB per ring: 2048-col heads on sync/gpsimd (8KB descriptors,
    # ~143GB/s/ring vs ~79 at 4KB), middle columns early on the scalar
    # ring (before wqf -- the scalar ring is the slowest, ~60-79GB/s),
    # 1024-col tails as the sync/gpsimd second chunks.
    ld(nc.sync, 0, 0, 2048)
    ld(nc.gpsimd, 1, 0, 2048)
    ld(nc.scalar, 0, 2048, 3072)
    ld(nc.scalar, 1, 2048, 3072)
    ld(nc.sync, 0, 3072, 4096)
    ld(nc.gpsimd, 1, 3072, 4096)
    wqf = small.tile([P, 2, 3 * C], F32, tag="wqkvTf", name="wqf")
    nc.scalar.dma_start(wqf, wqkvT_d[:, :, :])
    for w in range(WARMUP_MM):
        pw = ps_s.tile([P, P], F32, tag="s", name=f"warm{w}")
        nc.tensor.matmul(pw, lhsT=gmat_sb, rhs=gmat_sb, start=True, stop=True)
    mv = small.tile([P, 2, 2], F32, tag="mv")    # [:, ct, (mean, var)]
    for j in range(NB):
        for ct in range(NCT):
            csl = slice(j * NBLK, (j + 1) * NBLK)
            nc.vector.bn_stats(stats_t[ct][:, j, :], x_sb[:, ct, csl])
            nc.scalar.copy(x8[:, ct, csl], x_sb[:, ct, csl])
    for ct in range(NCT):
        nc.vector.bn_aggr(mv[:, ct], stats_t[ct])

    # ---- GN stats -> per-channel scale/shift (h = x*sc + sh). Both
    # channel-tiles ride as [P, 2]-column pairs through ONE chain of small
    # ops (the old per-ct chain serialized ~20 DVE ops at ~150ns each). ----
    # group-average + broadcast back to channels via PE; the mean matmul
    # fires straight off mv while the DVE computes E[x^2]. g4 layout:
    # cols 0-1 = gmean per ct, cols 2-3 = gE[x^2] per ct.
    psg = ps_s.tile([P, NBLK], F32, tag="s")
    nc.tensor.matmul(psg[:, 0:2], lhsT=gmat_sb, rhs=mv[:, :, 0:1],
                     start=True, stop=True)
    m2 = small.tile([P, 2], F32, tag="m2")
    nc.vector.tensor_mul(m2, mv[:, :, 0:1], mv[:, :, 0:1])
    nc.vector.tensor_add(m2, m2, mv[:, :, 1:2])
    nc.tensor.matmul(psg[:, 2:4], lhsT=gmat_sb, rhs=m2, start=True,
                     stop=True, skip_group_check=True)
    g4 = small.tile([P, 4], F32, tag="g4")
    nc.vector.tensor_copy(g4, psg[:, 0:4])
    # sc = gamma * rsqrt(gvar + eps);  sh = beta - gmean * sc
    tmp2 = small.tile([P, 2], F32, tag="tmp2")
    sc2 = small.tile([P, 2], F32, tag="sc2")
    sh2 = small.tile([P, 2], F32, tag="sh2")
    nc.vector.tensor_mul(tmp2, g4[:, 0:2], g4[:, 0:2])
    nc.vector.tensor_tensor(tmp2, g4[:, 2:4], tmp2, ALU.subtract)  # var
    nc.scalar.activation(tmp2, tmp2, AF.Sqrt, bias=eps_t)
    nc.vector.reciprocal(tmp2, tmp2)                                  # rstd
    nc.vector.tensor_mul(sc2, tmp2, consts_t[:, 0:2])
    nc.vector.tensor_mul(tmp2, g4[:, 0:2], sc2)
    nc.vector.tensor_tensor(sh2, consts_t[:, 2:4], tmp2, ALU.subtract)
    scale_sh = [(sc2[:, ct:ct + 1], sh2[:, ct:ct + 1]) for ct in range(NCT)]

    # ---- fold GN into the weights: wq8[c,:] = wqf[c,:] * sc[c] in fp8.
    # Staggered by consumer: the k columns fold first on DVE (they gate
    # the k projections, the very first post-stats matmuls), then vp,
    # while ACT folds the q columns in parallel. The exp-table switch
    # (1.28us implicit ACT load) is sequenced AFTER ACT's fold pieces so
    # it stays off the projection gate but before the first real exp. ----
    # dim-1 stride padded +16 cols: LDWEIGHTS from 768-stride wq8 slices
    # measured 370ns vs 134ns from the 272-stride vt tiles (SBUF access
    # pattern conflict); the pad mirrors vt's fast 256+16 layout
    wq8 = small.tile([P, 2, 3 * C + 16], FP8, tag="wqkvT8", name="wq8")
    for ct in range(NCT):
        nc.vector.tensor_scalar(wq8[:, ct, C:2 * C], wqf[:, ct, C:2 * C],
                                scale_sh[ct][0], None, op0=ALU.mult)
    for ct in range(NCT):
        nc.vector.tensor_scalar(wq8[:, ct, 2 * C:3 * C],
                                wqf[:, ct, 2 * C:3 * C],
                                scale_sh[ct][0], None, op0=ALU.mult)
    for ct in range(NCT):
        nc.scalar.activation(wq8[:, ct, 0:C], wqf[:, ct, 0:C], AF.Copy,
                             bias=0.0, scale=scale_sh[ct][0])
    # switch the ACT table back to Exp now, off the critical path, so the
    # first softmax exp doesn't pay the 1.28us implicit load
    nc.scalar.activation(sqrt_dummy, eps_t, AF.Exp, scale=1.0)
    # q bias' = W_q @ sh + b_q and bp_eff = bp + W_vp @ sh. Reuse the fp8
    # folded weights (fp32 wqf LDWEIGHTS cost 333ns each x8 on the
    # critical path): W @ sh = wq8 @ (16*sh/sc) / 16; the x16 keeps the
    # tiny sh/sc out of fp8's subnormal range. Emitted via emit_bias4()
    # AFTER the kv matmuls, which don't depend on it.
    shq8 = small.tile([P, 2], FP8, tag="shq8")
    rsc = small.tile([P, 2], F32, tag="rsc")
    nc.vector.reciprocal(rsc, sc2)
    nc.vector.tensor_mul(rsc, rsc, sh2)
    nc.vector.tensor_scalar(shq8, rsc, 16.0, None, op0=ALU.mult)
    bias4 = small.tile([P, 4], F32, tag="bias4")
    bias_q = [bias4[:, o:o + 1] for o in range(NCT)]
    bp_eff = [bias4[:, 2 + o:3 + o] for o in range(NCT)]

    def emit_bias4():
        psb = ps_s.tile([P, NBLK], F32, tag="s")
        for o in range(NCT):
            for ct in range(NCT):
                nc.tensor.matmul(psb[:, o:o + 1],
                                 lhsT=wq8[:, ct, o * P:(o + 1) * P],
                                 rhs=shq8[:, ct:ct + 1], start=(ct == 0),
                                 stop=(ct == NCT - 1))
            for ct in range(NCT):
                nc.tensor.matmul(
                    psb[:, 2 + o:3 + o],
                    lhsT=wq8[:, ct, 2 * C + o * P:2 * C + (o + 1) * P],
                    rhs=shq8[:, ct:ct + 1], start=(ct == 0),
                    stop=(ct == NCT - 1))
        nc.vector.tensor_scalar(bias4, psb[:, 0:4], 1.0 / 16.0, None,
                                op0=ALU.mult)
        nc.vector.tensor_add(bias4, bias4, consts_t[:, 4:8])

    # ---- qkv projections (GN pre-folded, so rhs is x8 directly). q/k land
    # in fp8 [128, 2, N] (channel-half on the middle dim) and vp in fp8
    # m-pair-interleaved [128, 2, 272] tiles so the attention matmuls can use
    # fp8 DoubleRow (2 values/PE-cell -> one matmul contracts 256). vp is the
    # w_proj-fused v projection; col 256 = ones (softmax denominators). ----
    q2 = big.tile([P, 2, N], FP8, tag="q2")
    k2 = big.tile([P, 2, N], FP8, tag="k2")
    VTW = 272
    vt_lo = big.tile([P, NM // 4, 2, VTW], FP8, tag="vt0", name="vt_lo")
    vt_hi = big.tile([P, NM // 4, 2, VTW], FP8, tag="vt1", name="vt_hi")
    # pre-fill the ones columns once (strided memsets, off-critical-path)
    nc.gpsimd.memset(vt_lo[:, :, :, C:C + 1], 1.0)
    nc.gpsimd.memset(vt_hi[:, :, :, C:C + 1], 1.0)

    def vt2(pair):
        return (vt_lo[:, pair] if pair < NM // 4
                else vt_hi[:, pair - NM // 4])

    def emit_q_blk(blk, only_o=None, eng=None):
        """q projection for block blk: 2 matmuls + 2 biased fp8 casts
        (ACT activation or DVE tensor_scalar; GPSIMD can't read PSUM).
        Deferrable (per channel-half) to just before block blk needs q2."""
        bsl = slice(blk * NBLK, (blk + 1) * NBLK)
        for o in range(NCT):
            if only_o is not None and o != only_o:
                continue
            ps = ps_s.tile([P, NBLK], F32, tag="s")
            nc.tensor.matmul(
                ps, lhsT=wq8[:, :, o * P:(o + 1) * P], rhs=x8[:, :, bsl],
                start=True, stop=True, perf_mode=DR)
            e = eng if eng is not None else (nc.scalar if o == 0 else
                                            nc.vector)
            if e is nc.scalar:
                nc.scalar.activation(q2[:, o, bsl], ps, AF.Identity,
                                     bias=bias_q[o], scale=1.0)
            else:
                e.tensor_scalar(q2[:, o, bsl], ps, bias_q[o], None,
                                op0=ALU.add)

    def kv_piece_k(blk):
        """k projection for block blk: 2 matmuls + 2 psum->fp8 casts (one
        ACT + one DVE, so neither engine sees more than one copy)."""
        bsl = slice(blk * NBLK, (blk + 1) * NBLK)
        for o in range(NCT):
            ps = ps_s.tile([P, NBLK], F32, tag="s")
            nc.tensor.matmul(
                ps, lhsT=wq8[:, :, C + o * P:C + (o + 1) * P],
                rhs=x8[:, :, bsl], start=True, stop=True, perf_mode=DR)
            if o == 0:
                nc.scalar.copy(k2[:, o, bsl], ps)
            else:
                nc.vector.tensor_copy(k2[:, o, bsl], ps)

    def kv_piece_vp(blk, half):
        """vp projection for pair 2*blk+half: 2 matmuls + 1 ACT + 1 DVE
        cast."""
        for m in (4 * blk + 2 * half, 4 * blk + 2 * half + 1):
            ps = ps_s.tile([P, NBLK], F32, tag="s")
            nc.tensor.matmul(
                ps[:, 0:C], lhsT=x8[:, :, m * P:(m + 1) * P],
                rhs=wq8[:, :, 2 * C:3 * C],
                start=True, stop=True, perf_mode=DR)
            dst = vt2(m // 2)[:, m % 2]
            if m % 2 == 0:
                nc.scalar.copy(dst[:, 0:C], ps[:, 0:C])
            else:
                nc.vector.tensor_copy(dst[:, 0:C], ps[:, 0:C])

    def emit_kv_blk(blk):
        kv_piece_k(blk)
        kv_piece_vp(blk, 0)
        kv_piece_vp(blk, 1)

    # ---- softmax divide + output helpers ----
    def emit_div_a(pend, csl=slice(0, NBLK)):
        pav0, pav1, psum, nb = pend
        w = csl.stop - csl.start
        sums_sb = work2.tile([1, NBLK], F32, tag="sums")
        nc.scalar.activation(sums_sb[:, csl], psum[:, csl], AF.Copy, bias=0.0)
        bc2 = work2.tile([P, NBLK], F32, tag="bc2")
        nc.gpsimd.partition_broadcast(bc2[:, csl], sums_sb[:, csl])
        bc_sb = work2.tile([P, NBLK], F32, tag="bc")
        nc.vector.reciprocal_approx_fast(bc_sb[:, csl], bc2[:, csl])
        return bc_sb

    def emit_div_b(pend, bc_sb, o, csl=slice(0, NBLK), dma_eng=None,
                   xbp=None):
        """One output channel-half: divide + bias + residual + store.
        Split into two calls so the DVE burst spreads across the block.
        With xbp (a pre-staged x+bp tile, last block only) the residual
        add runs on GPSIMD instead of the DVE."""
        pav = pend[o]
        nb = pend[3]
        nsl = slice(nb * NBLK + csl.start, nb * NBLK + csl.stop)
        t = tdiv.tile([P, NBLK], F32, tag="t")
        nc.vector.tensor_mul(t[:, csl], pav[:, csl], bc_sb[:, csl])
        st = stage.tile([P, NBLK], F32, tag="st")
        if xbp is not None:
            nc.gpsimd.tensor_tensor(st[:, csl], t[:, csl], xbp[:, csl],
                                    ALU.add)
        else:
            nc.vector.scalar_tensor_tensor(st[:, csl], t[:, csl], bp_eff[o],
                                           x_sb[:, o, nsl],
                                           op0=ALU.add, op1=ALU.add)
        if dma_eng is None:
            dma_eng = nc.sync if o == 0 else nc.gpsimd
        dma_eng.dma_start(out_d[o * P:(o + 1) * P, nsl], st[:, csl])

    # ---- global software-pipelined attention loop over gp = nb*16 + pair.
    # At iteration gp we emit: exps(gp+1) (engine queues run them while PE
    # works), the score matmuls for gp+2 (split around the avs so the bank
    # freed by exp0(gp+1) is reused late), and the av matmuls for gp (whose
    # e2 was exp'd during iteration gp-1 -> a full pair-period of exp slack,
    # so exp latency never stalls the PE). ----
    NPAIR = NM // 2          # 16 pairs per block
    NGP = NB * NPAIR         # 128
    ps_m = {}
    e2_pend = {}
    blk_tiles = {}

    def emit_scores(gp, half):
        if gp >= NGP:
            return
        nb, p = divmod(gp, NPAIR)
        m = 2 * p + half
        ps = ps_s.tile([P, NBLK], F32, tag="s")
        nc.tensor.matmul(ps, lhsT=k2[:, :, m * P:(m + 1) * P],
                         rhs=q2[:, :, nb * NBLK:(nb + 1) * NBLK],
                         start=True, stop=True, perf_mode=DR)
        ps_m[(gp, half)] = ps

    def emit_exps(gp):
        # Half-split: every pair's half0 exp on ACT (exact Exp) and half1
        # on DVE (EXP8 approx). One 686ns exp per engine per 1075ns gp --
        # perfectly periodic, so neither engine ever runs a 2.7us burst
        # that outlives the score->av window (runs-of-2 pairs were 400-
        # 700ns late on every second pair; single-pair runs had no queue
        # slack for the kv/q/div work and cascaded). Exception: the pairs
        # that coincide with the DVE's div work (p4, p8) go fully to ACT,
        # and the pairs next to ACT's sums/q copies (p2, p6) fully to
        # DVE, so each engine's extras land in its own exp-free slots.
        if gp >= NGP:
            return
        nb, p = divmod(gp, NPAIR)
        e2 = work.tile([P, 2, NBLK], FP8, tag="e")
        ps0 = ps_m.pop((gp, 0))
        ps1 = ps_m.pop((gp, 1))
        if nb > 0 and p in (4, 8):
            engs = ("act", "act")
        else:
            engs = ("act", "dve")
        for ps, eng, half in ((ps0, engs[0], 0), (ps1, engs[1], 1)):
            if eng == "act":
                nc.scalar.activation(e2[:, half], ps, AF.Exp,
                                     scale=float(SCALE))
            else:
                nc.vector._custom_dve(EXP8, out=e2[:, half], in0=ps,
                                      s0=EXP_C0, s1=_EXP_C1, imm2=_EXP_C2)
        e2_pend[gp] = e2

    def emit_avs(gp):
        nb, p = divmod(gp, NPAIR)
        e2 = e2_pend.pop(gp)
        pav0, pav1, psum = blk_tiles[nb]
        first, last = (p == 0), (p == NPAIR - 1)
        vtp = vt2(p)
        if gp == NGP - 1:
            # den matmul first on the very last gp: the tail's div chain
            # hangs off it, so finishing it 2 matmuls earlier matters
            nc.tensor.matmul(psum, lhsT=vtp[:, :, 2 * P:2 * P + 1], rhs=e2,
                             start=first, stop=last, perf_mode=DR)
        nc.tensor.matmul(pav0, lhsT=vtp[:, :, 0:P], rhs=e2,
                         start=first, stop=last, perf_mode=DR)
        nc.tensor.matmul(pav1, lhsT=vtp[:, :, P:2 * P], rhs=e2,
                         start=first, stop=last, perf_mode=DR)
        if gp != NGP - 1:
            nc.tensor.matmul(psum, lhsT=vtp[:, :, 2 * P:2 * P + 1], rhs=e2,
                             start=first, stop=last, perf_mode=DR)

    def new_blk_tiles(nb):
        pav0 = ps_av0.tile([P, NBLK], F32, tag="av0", name=f"av0_{nb}")
        pav1 = ps_av1.tile([P, NBLK], F32, tag="av1", name=f"av1_{nb}")
        psum = ps_sum.tile([1, NBLK], F32, tag="sum", name=f"sum_{nb}")
        blk_tiles[nb] = (pav0, pav1, psum)

    # Fused phase gating: kv block b unlocks k2 m-tiles < 4(b+1) and vt
    # pairs < 2(b+1); at iteration gp of block 0 the scores reach m-tile
    # 2*gp+5 and the avs read vt pair gp, both covered once kv blocks
    # <= gp/2 + 2 are in.
    # Pre-attention: ONLY what the first scores need -- kv(0), k(1), the
    # bias, q(0). Everything else streams through block 0's iterations
    # (2 pieces early, 1 later): the engines' copy backlog then overlaps
    # PE attention work instead of the PE idling ~5us while q(0)'s copies
    # sit behind 16 kv casts in the ACT/DVE queues.
    kv_piece_k(0)
    emit_bias4()
    emit_q_blk(0)
    kv_piece_vp(0, 0)
    kv_piece_k(1)
    kv_piece_vp(0, 1)
    kvq = []
    for b in range(1, NB):
        if b > 1:
            kvq.append(lambda b=b: kv_piece_k(b))
        kvq.append(lambda b=b: kv_piece_vp(b, 0))
        kvq.append(lambda b=b: kv_piece_vp(b, 1))
    emit_scores(0, 0)
    emit_scores(0, 1)
    emit_exps(0)
    emit_scores(1, 0)
    emit_scores(1, 1)

    state = {"pend": None, "bc_prev": None}
    for gp in range(NGP):
        nb, p = divmod(gp, NPAIR)
        if nb not in blk_tiles:
            new_blk_tiles(nb)
        emit_exps(gp + 1)
        emit_scores(gp + 2, 0)
        # kv pieces AFTER the exps: their psum->fp8 casts then queue
        # behind the gp's exp on ACT/DVE instead of head-of-line blocking
        # it while the kv matmul is still in the PE pipeline
        if nb == 0:
            for _ in range(2 if p < 6 else 1):
                if kvq:
                    kvq.pop(0)()
            # q(1) deferred past the early copy backlog (needed only by
            # block 1's scores, emitted from iteration 14)
            if p == 8:
                emit_q_blk(1)
            if p == 12:
                emit_q_blk(2)
        # div_b lands at p4/p8 -- the DVE's exp-free slots (those pairs'
        # exps both run on ACT), where the full-width mul+stt (~1.5us)
        # fits the 1.47us gp-plus-slack without chunking overhead
        if p == 4 and nb > 0:
            emit_div_b(state["pend"], state["bc_prev"], 0)
        if p == 8 and nb > 0:
            emit_div_b(state["pend"], state["bc_prev"], 1)
        if p == 2 and nb >= 2 and nb + 1 < NB:
            emit_q_blk(nb + 1, only_o=0, eng=nc.scalar)
        if p == 6 and nb >= 2 and nb + 1 < NB:
            emit_q_blk(nb + 1, only_o=1, eng=nc.vector)
        if nb == NB - 2 and p in (10, 14):
            # stage x+bp for the LAST block's residual during block 6
            # (ACT slack, far from block 7's critical last exps): the
            # tail's o=1 adds then run on GPSIMD instead of the DVE,
            # which otherwise serializes recip+mul+stt after the final
            # den matmul
            o7 = 0 if p == 10 else 1
            nsl7 = slice((NB - 1) * NBLK, NB * NBLK)
            t7 = stage.tile([P, NBLK], F32, tag=f"xbp7_{o7}")
            nc.scalar.activation(t7, x_sb[:, o7, nsl7], AF.Identity,
                                 bias=bp_eff[o7], scale=1.0)
            state[f"xbp7_{o7}"] = t7
        emit_avs(gp)
        emit_scores(gp + 2, 1)
        if p == NPAIR - 1 and nb < NB - 1:
            # div_a emitted before the next block's first den matmul can
            # touch the single-buffered ps_sum bank
            state["pend"] = (*blk_tiles.pop(nb), nb)
            state["bc_prev"] = emit_div_a(state["pend"])
    # ---- tail: the last block's divide runs with nothing behind it, so
    # chunk it in half-width pieces pipelined across ACT/GPS/DVE and spread
    # the final stores over four DMA queues. ----
    pend = (*blk_tiles.pop(NB - 1), NB - 1)
    HB = NBLK // 2
    csls = [slice(0, HB), slice(HB, NBLK)]
    bcs = [emit_div_a(pend, csl) for csl in csls]
    # GPS takes 3 of the 4 residual adds (from the block-6-staged x+bp
    # tiles); the GPS-bound muls are emitted first so the adds start
    # while the DVE works its own stt
    emit_div_b(pend, bcs[0], 1, csls[0], dma_eng=nc.gpsimd,
               xbp=state["xbp7_1"])
    emit_div_b(pend, bcs[1], 1, csls[1], dma_eng=nc.gpsimd,
               xbp=state["xbp7_1"])
    emit_div_b(pend, bcs[0], 0, csls[0], dma_eng=nc.sync)
    emit_div_b(pend, bcs[1], 0, csls[1], dma_eng=nc.sync,
               xbp=state["xbp7_0"])


def build_nc() -> bass.Bass:
    nc = bacc.Bacc("TRN2", target_bir_lowering=False, debug=False)
    x = nc.dram_tensor("x", [C, N], F32, kind="ExternalInput")
    wqkvT = nc.dram_tensor("wqkvT", [P, 2, 3 * C], F32, kind="ExternalInput")
    consts = nc.dram_tensor("consts", [P, 8], F32, kind="ExternalInput")
    gmat = nc.dram_tensor("gmat", [P, P], F32, kind="ExternalInput")
    out = nc.dram_tensor("out", [C, N], F32, kind="ExternalOutput")
    with tile.TileContext(nc) as tc:
        emit_kernel(tc, out.ap(), x.ap(), wqkvT.ap(), consts.ap(), gmat.ap())
    nc.compile()
    return nc


_NC_CACHE: list = []


def _in_maps(x, gamma, beta, w_qkv, b_qkv, w_proj, b_proj):
    f = lambda a: np.ascontiguousarray(np.asarray(a, dtype=np.float32))
    xs = f(x).reshape(B, C, N)
    w_qkv = np.asarray(w_qkv, dtype=np.float64)
    w_proj = np.asarray(w_proj, dtype=np.float64)
    b_qkv = np.asarray(b_qkv, dtype=np.float64)
    b_proj = np.asarray(b_proj, dtype=np.float64)
    # fuse w_proj into the v projection; its bias rides into bproj (softmax
    # rows sum to 1, so a constant vp offset is a constant output offset)
    w_fused = np.concatenate(
        [w_qkv[0:2 * C], w_proj @ w_qkv[2 * C:3 * C]], axis=0)
    bp_eff = b_proj + w_proj @ b_qkv[2 * C:3 * C]
    # consts cols: gamma0 gamma1 beta0 beta1 bq0 bq1 bp0 bp1
    gamma = np.asarray(gamma, np.float64)
    beta = np.asarray(beta, np.float64)
    consts = np.stack(
        [gamma[0:P], gamma[P:C], beta[0:P], beta[P:C],
         b_qkv[0:P], b_qkv[P:C], bp_eff[0:P], bp_eff[P:C]], axis=1)
    base = {
        "wqkvT": f(w_fused.T.reshape(2, P, 3 * C).transpose(1, 0, 2)),
        "consts": f(consts),
        "gmat": _group_mat(),
    }
    return [{**base, "x": np.ascontiguousarray(xs[i])} for i in range(B)]


def run_spmd(x, gamma, beta, w_qkv, b_qkv, w_proj, b_proj, **kwargs):
    from concourse.bass_utils import run_bass_kernel_spmd

    if not _NC_CACHE:
        _NC_CACHE.append(build_nc())
    nc = _NC_CACHE[0]
    maps = _in_maps(x, gamma, beta, w_qkv, b_qkv, w_proj, b_proj)
    res = run_bass_kernel_spmd(nc, maps, core_ids=list(range(B)), **kwargs)
    out = np.stack([res.results[i]["out"] for i in range(B)])
    return out.reshape(B, C, H, W), res


def kernel(x, gamma, beta, w_qkv, b_qkv, w_proj, b_proj) -> np.ndarray:
    out, _ = run_spmd(x, gamma, beta, w_qkv, b_qkv, w_proj, b_proj)
    return out

